# revision 31
# baseline (speedup 1.0000x reference)
"""Trainium2 Bass kernel for nn_MultiHeadAttention (B=4, L=S=2048, D=1024, H=16, causal).

Sharding: 8 cores = 4 batches x 2 head-groups (8 heads each).
Per core: project its batch's q/k/v against its group's weight slices,
causal attention for 8 heads, output-projection against Wo column slice.
Host sums the 2 partial outputs per batch (tensor-parallel reduce).

v2 layout notes:
- All input transposes happen on the host (numpy), so every device DMA is
  linear. X arrives as xT [D, T] per tensor; weights pre-transposed.
- Softmax sums are folded into the PV matmul: each head's V tile carries a
  65th column of ones, so ctx PSUM row 64 accumulates sum(P) for free.
- Scores for diagonal blocks are tightened to the causal width; the mask
  multiply zeroes the stale/garbage columns (masks have 0 there).
- Projections, attention and Wo are interleaved chunk-by-chunk so the PE
  instruction stream never drains.

All matmuls bf16 with fp32 PSUM accumulation.
"""

import sys

if "/opt/trn_rl_repo" not in sys.path:
    sys.path.insert(0, "/opt/trn_rl_repo")

import numpy as np
import ml_dtypes

BF16 = ml_dtypes.bfloat16

# Problem constants (hardcoded per harness contract)
B, L, D, H = 4, 2048, 1024, 16
HD = D // H              # 64
NCORES = 8
GROUPS = 2               # head-groups (tensor parallel)
HG = H // GROUPS         # 8 heads per group
DG = HG * HD             # 512 out-dim per group

T = L                    # tokens per core
DM = D                   # model dim
NDC = DM // 128          # 8 contraction chunks
NP = HG // 2             # 4 head pairs
NCH = T // 512           # 4 token chunks
LCH = 512
TB = 128
NT = T // TB             # 16 token tiles
SCALE = 1.0 / np.sqrt(HD)

FULL_CFG = dict(T=T, DM=DM, DG=DG)


def emit_mha(tc, aps):
    import concourse.bass as bass
    from concourse import mybir

    nc = tc.nc
    f32 = mybir.dt.float32
    bf16 = mybir.dt.bfloat16
    Exp = mybir.ActivationFunctionType.Exp

    import contextlib

    ctx = contextlib.ExitStack()
    with ctx:
        wts = ctx.enter_context(tc.tile_pool(name="wts", bufs=1))
        xpool = ctx.enter_context(tc.tile_pool(name="xp", bufs=2))
        vt_pool = ctx.enter_context(tc.tile_pool(name="vt", bufs=1))
        qt_pool = ctx.enter_context(tc.tile_pool(name="qt", bufs=1))
        kt_pool = ctx.enter_context(tc.tile_pool(name="kt", bufs=1))
        pt_pool = ctx.enter_context(tc.tile_pool(name="ptp", bufs=4))
        cpc_pool = ctx.enter_context(tc.tile_pool(name="cpc", bufs=1))
        ctxn_pool = ctx.enter_context(tc.tile_pool(name="ctxn", bufs=2))
        small = ctx.enter_context(tc.tile_pool(name="small", bufs=1))
        osb_pool = ctx.enter_context(tc.tile_pool(name="osb", bufs=2))
        # PSUM: scores 2x2 banks + ctxA 1 + ctxB 1 + proj 2 = 8 banks
        st_ps = ctx.enter_context(tc.tile_pool(name="st_ps", bufs=2, space="PSUM"))
        ctx_ps = ctx.enter_context(tc.tile_pool(name="ctx_ps", bufs=1, space="PSUM"))
        proj_ps = ctx.enter_context(tc.tile_pool(name="proj_ps", bufs=2, space="PSUM"))

        # ---- weight/mask DMAs on the Activation HWDGE queue ----
        wvT, wqT, wkT = [], [], []
        for nm, lst in (("wv", wvT), ("wq", wqT), ("wk", wkT)):
            for c in range(NDC):
                t = wts.tile([128, DG], bf16, tag=f"{nm}{c}", name=f"{nm}{c}")
                nc.scalar.dma_start(out=t[:], in_=aps[nm + "T"][c * 128:(c + 1) * 128, :])
                lst.append(t)
        woT2 = []
        for p in range(NP):
            t = wts.tile([128, DM], bf16, tag=f"wo{p}", name=f"wo{p}")
            nc.scalar.dma_start(out=t[:], in_=aps["woT"][p * 128:(p + 1) * 128, :])
            woT2.append(t)
        masks = []
        for r in range(4):
            mt = wts.tile([TB, 2 * LCH], bf16, tag=f"mask{r}", name=f"mask{r}")
            nc.scalar.dma_start(out=mt[:], in_=aps["maskt"][r])
            masks.append(mt)

        # pre-zero the pt buffers (stale cols are mask-multiplied; NaN*0=NaN)
        pt_boot = []
        for _ in range(4):
            pt = pt_pool.tile([128, 2 * LCH], bf16, tag="pt")
            nc.vector.memset(pt[:], 0.0)
            pt_boot.append(pt)
        # pre-zero score PSUM so exp of untouched columns stays finite
        for _ in range(2):
            sp = st_ps.tile([128, 2 * LCH], f32, tag="st")
            nc.vector.memset(sp[:], 0.0)

        vt = [None] * NT            # [128, HG, HD+1] V tiles (ones in col HD)
        QT = [[None] * NCH for _ in range(NP)]
        KT = [[None] * NCH for _ in range(NP)]

        # ---- filler queues: projection/Wo matmuls emitted one-at-a-time
        # between attention blocks so the PE stream never drains ----
        fq_proj, fq_wo = [], []
        state = {"blk": 0, "popped": 0, "appended": 0}

        def push_proj(fn):
            fq_proj.append(fn)
            state["appended"] += 1

        def draw(k):
            for _ in range(k):
                if fq_proj:
                    fq_proj.pop(0)()
                    state["popped"] += 1
                elif fq_wo and state["blk"] >= 8:
                    fq_wo.pop(0)()
                else:
                    return

        def flush(q):
            while q:
                q.pop(0)()
                if q is fq_proj:
                    state["popped"] += 1

        def flush_to(marker):
            while state["popped"] < marker:
                fq_proj.pop(0)()
                state["popped"] += 1

        def gen_v(n, xv_n):
            for stl in range(4):
                st = 4 * n + stl
                h = {}
                for c in range(NDC):
                    def vop(c=c, stl=stl, st=st, h=h, xv_n=xv_n):
                        if c == 0:
                            h["ps"] = proj_ps.tile([128, DG], f32, tag="proj", name="pps")
                        nc.tensor.matmul(h["ps"][:],
                                         lhsT=xv_n[c][:, stl * 128:(stl + 1) * 128],
                                         rhs=wvT[c][:], start=(c == 0),
                                         stop=(c == NDC - 1))
                        if c == NDC - 1:
                            v = vt_pool.tile([128, HG, HD + 1], bf16, tag=f"V{st}",
                                             name=f"V{st}")
                            nc.vector.tensor_copy(
                                v[:, :, 0:HD],
                                h["ps"][:].rearrange("a (b c) -> a b c", b=HG))
                            nc.vector.memset(v[:, :, HD:HD + 1], 1.0)
                            vt[st] = v
                    push_proj(vop)

        def gen_qk(n, p, xq_n, xk_n):
                hq, hk = {}, {}
                for c in range(NDC):
                    def qop(c=c, p=p, n=n, h=hq, xq_n=xq_n):
                        if c == 0:
                            h["ps"] = proj_ps.tile([128, LCH], f32, tag="proj", name="pps")
                        nc.tensor.matmul(h["ps"][:],
                                         lhsT=wqT[c][:, p * 128:(p + 1) * 128],
                                         rhs=xq_n[c][:], start=(c == 0),
                                         stop=(c == NDC - 1))
                        if c == NDC - 1:
                            qt = qt_pool.tile([128, LCH], bf16, tag=f"QT{p}_{n}",
                                              name=f"QT{p}_{n}")
                            nc.vector.tensor_copy(qt[:], h["ps"][:])
                            QT[p][n] = qt
                    push_proj(qop)
                for c in range(NDC):
                    def kop(c=c, p=p, n=n, h=hk, xk_n=xk_n):
                        if c == 0:
                            h["ps"] = proj_ps.tile([128, LCH], f32, tag="proj", name="pps")
                        nc.tensor.matmul(h["ps"][:],
                                         lhsT=wkT[c][:, p * 128:(p + 1) * 128],
                                         rhs=xk_n[c][:], start=(c == 0),
                                         stop=(c == NDC - 1))
                        if c == NDC - 1:
                            kt = kt_pool.tile([128, LCH], bf16, tag=f"KT{p}_{n}",
                                              name=f"KT{p}_{n}")
                            nc.vector.tensor_copy(kt[:], h["ps"][:])
                            KT[p][n] = kt
                    push_proj(kop)

        def gen_wo(m, ctxn):
            for ltl in range(4):
                lt = 4 * m + ltl
                h = {}
                for oc in range(2):
                    for p in range(NP):
                        def wop(ltl=ltl, lt=lt, oc=oc, p=p, h=h, ctxn=ctxn):
                            if oc == 0 and p == 0:
                                h["osb"] = osb_pool.tile([128, DM], f32, tag="osb", name="osb")
                            if p == 0:
                                h["ps"] = proj_ps.tile([128, LCH], f32, tag="proj", name="pps")
                            nc.tensor.matmul(
                                h["ps"][:],
                                lhsT=ctxn[p][:, ltl * 128:(ltl + 1) * 128],
                                rhs=woT2[p][:, oc * LCH:(oc + 1) * LCH],
                                start=(p == 0), stop=(p == NP - 1))
                            if p == NP - 1:
                                nc.vector.tensor_copy(
                                    h["osb"][:, oc * LCH:(oc + 1) * LCH], h["ps"][:])
                                # split the writeback across both HWDGE queues
                                eng = nc.sync if oc == 0 else nc.scalar
                                eng.dma_start(
                                    out=aps["y"][lt * TB:(lt + 1) * TB,
                                                 oc * LCH:(oc + 1) * LCH],
                                    in_=h["osb"][:, oc * LCH:(oc + 1) * LCH])
                        fq_wo.append(wop)

        def attn_pair(p, i, prev_finish):
            """Causal attention for head-pair p over l-chunk i.

            Returns (ctxn_tile, finish_fn): finish_fn emits the deferred
            normalize multiplies (call it a few blocks into the NEXT pair so
            the broadcast DMA has completed before the DVE reaches them).
            """
            jmax = 4 * i + 3
            QTi = QT[p][i]
            ctxA = ctx_ps.tile([65, LCH], f32, tag="ctxA")
            ctxB = ctx_ps.tile([65, LCH], f32, tag="ctxB")
            pts = {}

            def sc_act(j):
                r = j - 4 * i
                off = 128 * r if r > 0 else 0
                sp = st_ps.tile([128, 2 * LCH], f32, tag="st")
                jn, jo = j // 4, (j % 4) * 128
                KTj = KT[p][jn]
                nc.tensor.matmul(sp[:, off:LCH], lhsT=KTj[0:64, jo:jo + 128],
                                 rhs=QTi[0:64, off:LCH], start=True, stop=True)
                nc.tensor.matmul(sp[:, LCH + off:2 * LCH], lhsT=KTj[64:128, jo:jo + 128],
                                 rhs=QTi[64:128, off:LCH], start=True, stop=True)
                pt = pt_pool.tile([128, 2 * LCH], bf16, tag="pt")
                nc.scalar.activation(pt[:, off:2 * LCH], sp[:, off:2 * LCH], Exp,
                                     scale=float(SCALE))
                if r >= 0:
                    # zero the causal triangle; PV never reads cols < off, so
                    # only the 128-wide diagonal slice needs masking
                    nc.vector.tensor_mul(pt[:, off:off + 128], pt[:, off:off + 128],
                                         masks[r][:, off:off + 128])
                    nc.vector.tensor_mul(pt[:, LCH + off:LCH + off + 128],
                                         pt[:, LCH + off:LCH + off + 128],
                                         masks[r][:, LCH + off:LCH + off + 128])
                pts[j] = (pt, off)

            def pv(j):
                pt, off = pts.pop(j)
                st_f = (j == 0)
                en = (j == jmax)
                nc.tensor.matmul(ctxA[:, off:LCH], lhsT=vt[j][:, 2 * p, :],
                                 rhs=pt[:, off:LCH], start=st_f, stop=en,
                                 skip_group_check=True)
                nc.tensor.matmul(ctxB[:, off:LCH], lhsT=vt[j][:, 2 * p + 1, :],
                                 rhs=pt[:, LCH + off:2 * LCH], start=st_f, stop=en,
                                 skip_group_check=True)

            sc_act(0)
            for j in range(jmax + 1):
                if j + 1 <= jmax:
                    sc_act(j + 1)
                draw(2)
                pv(j)
                state["blk"] += 1
                draw(2)
                if j == 2 and prev_finish is not None:
                    prev_finish()
                    prev_finish = None
            # drain ctx+sums into one SBUF tile (A cols 0:LCH, B cols LCH:)
            cpc = cpc_pool.tile([65, 2 * LCH], f32, tag=f"cpc{p}", name=f"cpc{p}")
            nc.vector.tensor_copy(cpc[:, 0:LCH], ctxA[:])
            nc.vector.tensor_copy(cpc[:, LCH:2 * LCH], ctxB[:])
            # move the sums row to partition 0 (HW partition_broadcast reads
            # partition 0 only), approx-recip there, broadcast to all partitions
            rec = small.tile([1, 2 * LCH], f32, tag="rec")
            nc.scalar.dma_start(out=rec[:], in_=cpc[64:65, :])
            recr = small.tile([1, 2 * LCH], f32, tag="recr")
            nc.vector.reciprocal_approx_fast(out=recr[:], in_=rec[:])
            rb = small.tile([128, 2 * LCH], f32, tag="rb")
            nc.gpsimd.partition_broadcast(rb[:], recr[0:1, :])
            tn = ctxn_pool.tile([128, LCH], bf16, tag=f"cn{p}", name=f"cn{p}")

            def finish():
                tB = small.tile([64, LCH], bf16, tag="cnBtmp")
                nc.vector.tensor_mul(tn[0:64, :], cpc[0:64, 0:LCH], rb[0:64, 0:LCH])
                nc.vector.tensor_mul(tB[:], cpc[0:64, LCH:2 * LCH],
                                     rb[0:64, LCH:2 * LCH])
                nc.scalar.dma_start(out=tn[64:128, :], in_=tB[:])

            return tn, finish

        def emit_x_dmas(n):
            xs = []
            for nm, ap in (("xv", aps["xvT"]), ("xq", aps["xqT"]), ("xk", aps["xkT"])):
                lst = []
                for c in range(NDC):
                    t = xpool.tile([128, LCH], bf16, tag=f"{nm}{c}", name=f"{nm}{c}_{n}")
                    nc.sync.dma_start(out=t[:], in_=ap[c * 128:(c + 1) * 128,
                                                       n * LCH:(n + 1) * LCH])
                    lst.append(t)
                xs.append(lst)
            return xs

        ctxn_prev = None
        prev_finish = None
        # chunk 0: V + pair-0 Q/K upfront, remaining pairs' Q/K as filler
        # gated per pair (spreads the startup lump to dodge the HAM throttle)
        xs0 = emit_x_dmas(0)
        gen_v(0, xs0[0])
        gen_qk(0, 0, xs0[1], xs0[2])
        flush(fq_proj)
        markers0 = {}
        for p in range(1, NP):
            gen_qk(0, p, xs0[1], xs0[2])
            markers0[p] = state["appended"]
        for n in range(NCH):
            # queue next chunk's projections + previous chunk's Wo as filler
            if n + 1 < NCH:
                xs = emit_x_dmas(n + 1)
                gen_v(n + 1, xs[0])
                for p in range(NP):
                    gen_qk(n + 1, p, xs[1], xs[2])
            if n > 0:
                gen_wo(n - 1, ctxn_prev)
            state["blk"] = 0
            ctxn_cur = [None] * NP
            for p in range(NP):
                if n == 0 and p > 0:
                    flush_to(markers0[p])
                ctxn_cur[p], prev_finish = attn_pair(p, n, prev_finish)
            flush(fq_proj)
            flush(fq_wo)
            ctxn_prev = ctxn_cur

        if prev_finish is not None:
            prev_finish()
        gen_wo(NCH - 1, ctxn_prev)
        flush(fq_wo)


def make_mask_tiles(cfg):
    T_, LCH_, TB_ = cfg["T"], min(512, cfg["T"]), 128
    nMask = LCH_ // TB_
    f = np.arange(2 * LCH_) % LCH_
    p = np.arange(TB_)
    tiles = []
    for r in range(nMask):
        m = (f[None, :] >= (TB_ * r + p)[:, None]).astype(np.float32)
        tiles.append(m)
    return np.stack(tiles).astype(BF16)


def build_nc(cfg):
    """Build and compile the per-core Bass program. Returns nc."""
    import concourse.bacc as bacc
    import concourse.tile as tile
    from concourse import mybir

    T_, DM_, DG_ = cfg["T"], cfg["DM"], cfg["DG"]

    nc = bacc.Bacc("TRN2", target_bir_lowering=False, debug=False)
    f32 = mybir.dt.float32
    bf16 = mybir.dt.bfloat16
    aps = {}
    for nm, shape, dt in [
        ("xqT", [DM_, T_], bf16), ("xkT", [DM_, T_], bf16), ("xvT", [DM_, T_], bf16),
        ("wqT", [DM_, DG_], bf16), ("wkT", [DM_, DG_], bf16), ("wvT", [DM_, DG_], bf16),
        ("woT", [DG_, DM_], bf16),
        ("maskt", [4, 128, 1024], bf16),
    ]:
        aps[nm] = nc.dram_tensor(nm, shape, dt, kind="ExternalInput").ap()
    aps["y"] = nc.dram_tensor("y", [T_, DM_], f32, kind="ExternalOutput").ap()

    with tile.TileContext(nc) as tc:
        emit_mha(tc, aps)
    nc.compile()
    return nc


_CACHE = {}


def _get_nc():
    if "nc" not in _CACHE:
        _CACHE["nc"] = build_nc(FULL_CFG)
    return _CACHE["nc"]


def shard_inputs(q, k, v, Wq, Wk, Wv, Wo):
    """Build the per-core input maps (8 cores = 4 batches x 2 groups)."""
    maskt = make_mask_tiles(FULL_CFG)
    xT = {}
    for b in range(B):
        xT[b] = (np.ascontiguousarray(q[b].T).astype(BF16),
                 np.ascontiguousarray(k[b].T).astype(BF16),
                 np.ascontiguousarray(v[b].T).astype(BF16))
    wT = {}
    for g in range(GROUPS):
        rows = slice(g * DG, (g + 1) * DG)
        wT[g] = (np.ascontiguousarray(Wq[rows].T).astype(BF16),
                 np.ascontiguousarray(Wk[rows].T).astype(BF16),
                 np.ascontiguousarray(Wv[rows].T).astype(BF16),
                 np.ascontiguousarray(Wo[:, rows].T).astype(BF16))
    in_maps = []
    for core in range(NCORES):
        b, g = divmod(core, GROUPS)
        xqT, xkT, xvT = xT[b]
        wqT, wkT, wvT, woT = wT[g]
        in_maps.append({
            "xqT": xqT, "xkT": xkT, "xvT": xvT,
            "wqT": wqT, "wkT": wkT, "wvT": wvT, "woT": woT,
            "maskt": maskt,
        })
    return in_maps


def kernel(q, k, v, mask, Wq, Wk, Wv, Wo):
    from concourse import bass_utils

    q = np.asarray(q, dtype=np.float32)
    k = np.asarray(k, dtype=np.float32)
    v = np.asarray(v, dtype=np.float32)
    Wq = np.asarray(Wq, dtype=np.float32)
    Wk = np.asarray(Wk, dtype=np.float32)
    Wv = np.asarray(Wv, dtype=np.float32)
    Wo = np.asarray(Wo, dtype=np.float32)

    nc = _get_nc()
    in_maps = shard_inputs(q, k, v, Wq, Wk, Wv, Wo)
    res = bass_utils.run_bass_kernel_spmd(nc, in_maps, core_ids=list(range(NCORES)))
    out = np.zeros((B, L, D), dtype=np.float32)
    for core in range(NCORES):
        b = core // GROUPS
        out[b] += res.results[core]["y"]
    return out


# revision 32
# speedup vs baseline: 1.0018x; 1.0018x over previous
"""Trainium2 Bass kernel for nn_MultiHeadAttention (B=4, L=S=2048, D=1024, H=16, causal).

Sharding: 8 cores = 4 batches x 2 head-groups (8 heads each).
Per core: project its batch's q/k/v against its group's weight slices,
causal attention for 8 heads, output-projection against Wo column slice.
Host sums the 2 partial outputs per batch (tensor-parallel reduce).

v2 layout notes:
- All input transposes happen on the host (numpy), so every device DMA is
  linear. X arrives as xT [D, T] per tensor; weights pre-transposed.
- Softmax sums are folded into the PV matmul: each head's V tile carries a
  65th column of ones, so ctx PSUM row 64 accumulates sum(P) for free.
- Scores for diagonal blocks are tightened to the causal width; the mask
  multiply zeroes the stale/garbage columns (masks have 0 there).
- Projections, attention and Wo are interleaved chunk-by-chunk so the PE
  instruction stream never drains.

All matmuls bf16 with fp32 PSUM accumulation.
"""

import sys

if "/opt/trn_rl_repo" not in sys.path:
    sys.path.insert(0, "/opt/trn_rl_repo")

import numpy as np
import ml_dtypes

BF16 = ml_dtypes.bfloat16

# Problem constants (hardcoded per harness contract)
B, L, D, H = 4, 2048, 1024, 16
HD = D // H              # 64
NCORES = 8
GROUPS = 2               # head-groups (tensor parallel)
HG = H // GROUPS         # 8 heads per group
DG = HG * HD             # 512 out-dim per group

T = L                    # tokens per core
DM = D                   # model dim
NDC = DM // 128          # 8 contraction chunks
NP = HG // 2             # 4 head pairs
NCH = T // 512           # 4 token chunks
LCH = 512
TB = 128
NT = T // TB             # 16 token tiles
SCALE = 1.0 / np.sqrt(HD)

FULL_CFG = dict(T=T, DM=DM, DG=DG)


def emit_mha(tc, aps):
    import concourse.bass as bass
    from concourse import mybir

    nc = tc.nc
    f32 = mybir.dt.float32
    bf16 = mybir.dt.bfloat16
    Exp = mybir.ActivationFunctionType.Exp

    import contextlib

    ctx = contextlib.ExitStack()
    with ctx:
        wts = ctx.enter_context(tc.tile_pool(name="wts", bufs=1))
        xpool = ctx.enter_context(tc.tile_pool(name="xp", bufs=2))
        vt_pool = ctx.enter_context(tc.tile_pool(name="vt", bufs=1))
        qt_pool = ctx.enter_context(tc.tile_pool(name="qt", bufs=1))
        kt_pool = ctx.enter_context(tc.tile_pool(name="kt", bufs=1))
        pt_pool = ctx.enter_context(tc.tile_pool(name="ptp", bufs=4))
        cpc_pool = ctx.enter_context(tc.tile_pool(name="cpc", bufs=1))
        ctxn_pool = ctx.enter_context(tc.tile_pool(name="ctxn", bufs=2))
        small = ctx.enter_context(tc.tile_pool(name="small", bufs=1))
        osb_pool = ctx.enter_context(tc.tile_pool(name="osb", bufs=2))
        # PSUM: scores 2x2 banks + ctxA 1 + ctxB 1 + proj 2 = 8 banks
        st_ps = ctx.enter_context(tc.tile_pool(name="st_ps", bufs=2, space="PSUM"))
        ctx_ps = ctx.enter_context(tc.tile_pool(name="ctx_ps", bufs=1, space="PSUM"))
        proj_ps = ctx.enter_context(tc.tile_pool(name="proj_ps", bufs=2, space="PSUM"))

        # ---- weight/mask DMAs on the Activation HWDGE queue ----
        wvT, wqT, wkT = [], [], []
        for nm, lst in (("wv", wvT), ("wq", wqT), ("wk", wkT)):
            for c in range(NDC):
                t = wts.tile([128, DG], bf16, tag=f"{nm}{c}", name=f"{nm}{c}")
                nc.scalar.dma_start(out=t[:], in_=aps[nm + "T"][c * 128:(c + 1) * 128, :])
                lst.append(t)
        woT2 = []
        for p in range(NP):
            t = wts.tile([128, DM], bf16, tag=f"wo{p}", name=f"wo{p}")
            nc.scalar.dma_start(out=t[:], in_=aps["woT"][p * 128:(p + 1) * 128, :])
            woT2.append(t)
        masks = []
        for r in range(4):
            mt = wts.tile([TB, 2 * LCH], bf16, tag=f"mask{r}", name=f"mask{r}")
            nc.scalar.dma_start(out=mt[:], in_=aps["maskt"][r])
            masks.append(mt)

        # pre-zero the pt buffers (stale cols are mask-multiplied; NaN*0=NaN)
        pt_boot = []
        for _ in range(4):
            pt = pt_pool.tile([128, 2 * LCH], bf16, tag="pt")
            nc.vector.memset(pt[:], 0.0)
            pt_boot.append(pt)
        # pre-zero score PSUM so exp of untouched columns stays finite
        for _ in range(2):
            sp = st_ps.tile([128, 2 * LCH], f32, tag="st")
            nc.vector.memset(sp[:], 0.0)

        vt = [None] * NT            # [128, HG, HD+1] V tiles (ones in col HD)
        QT = [[None] * NCH for _ in range(NP)]
        KT = [[None] * NCH for _ in range(NP)]

        # ---- filler queues: projection/Wo matmuls emitted one-at-a-time
        # between attention blocks so the PE stream never drains ----
        fq_proj, fq_wo = [], []
        state = {"blk": 0, "popped": 0, "appended": 0}

        def push_proj(fn):
            fq_proj.append(fn)
            state["appended"] += 1

        def draw(k):
            for _ in range(k):
                if fq_proj:
                    fq_proj.pop(0)()
                    state["popped"] += 1
                elif fq_wo and state["blk"] >= 8:
                    fq_wo.pop(0)()
                else:
                    return

        def flush(q):
            while q:
                q.pop(0)()
                if q is fq_proj:
                    state["popped"] += 1

        def flush_to(marker):
            while state["popped"] < marker:
                fq_proj.pop(0)()
                state["popped"] += 1

        def gen_v(n, xv_n):
            for stl in range(4):
                st = 4 * n + stl
                h = {}
                for c in range(NDC):
                    def vop(c=c, stl=stl, st=st, h=h, xv_n=xv_n):
                        if c == 0:
                            h["ps"] = proj_ps.tile([128, DG], f32, tag="proj", name="pps")
                        nc.tensor.matmul(h["ps"][:],
                                         lhsT=xv_n[c][:, stl * 128:(stl + 1) * 128],
                                         rhs=wvT[c][:], start=(c == 0),
                                         stop=(c == NDC - 1))
                        if c == NDC - 1:
                            v = vt_pool.tile([128, HG, HD + 1], bf16, tag=f"V{st}",
                                             name=f"V{st}")
                            nc.vector.tensor_copy(
                                v[:, :, 0:HD],
                                h["ps"][:].rearrange("a (b c) -> a b c", b=HG))
                            nc.vector.memset(v[:, :, HD:HD + 1], 1.0)
                            vt[st] = v
                    push_proj(vop)

        def gen_qk(n, p, xq_n, xk_n):
                hq, hk = {}, {}
                for c in range(NDC):
                    def qop(c=c, p=p, n=n, h=hq, xq_n=xq_n):
                        if c == 0:
                            h["ps"] = proj_ps.tile([128, LCH], f32, tag="proj", name="pps")
                        nc.tensor.matmul(h["ps"][:],
                                         lhsT=wqT[c][:, p * 128:(p + 1) * 128],
                                         rhs=xq_n[c][:], start=(c == 0),
                                         stop=(c == NDC - 1))
                        if c == NDC - 1:
                            qt = qt_pool.tile([128, LCH], bf16, tag=f"QT{p}_{n}",
                                              name=f"QT{p}_{n}")
                            nc.vector.tensor_copy(qt[:], h["ps"][:])
                            QT[p][n] = qt
                    push_proj(qop)
                for c in range(NDC):
                    def kop(c=c, p=p, n=n, h=hk, xk_n=xk_n):
                        if c == 0:
                            h["ps"] = proj_ps.tile([128, LCH], f32, tag="proj", name="pps")
                        nc.tensor.matmul(h["ps"][:],
                                         lhsT=wkT[c][:, p * 128:(p + 1) * 128],
                                         rhs=xk_n[c][:], start=(c == 0),
                                         stop=(c == NDC - 1))
                        if c == NDC - 1:
                            kt = kt_pool.tile([128, LCH], bf16, tag=f"KT{p}_{n}",
                                              name=f"KT{p}_{n}")
                            nc.vector.tensor_copy(kt[:], h["ps"][:])
                            KT[p][n] = kt
                    push_proj(kop)

        def gen_wo(m, ctxn):
            for ltl in range(4):
                lt = 4 * m + ltl
                h = {}
                for oc in range(2):
                    for p in range(NP):
                        def wop(ltl=ltl, lt=lt, oc=oc, p=p, h=h, ctxn=ctxn):
                            if oc == 0 and p == 0:
                                h["osb"] = osb_pool.tile([128, DM], f32, tag="osb", name="osb")
                            if p == 0:
                                h["ps"] = proj_ps.tile([128, LCH], f32, tag="proj", name="pps")
                            nc.tensor.matmul(
                                h["ps"][:],
                                lhsT=ctxn[p][:, ltl * 128:(ltl + 1) * 128],
                                rhs=woT2[p][:, oc * LCH:(oc + 1) * LCH],
                                start=(p == 0), stop=(p == NP - 1))
                            if p == NP - 1:
                                nc.vector.tensor_copy(
                                    h["osb"][:, oc * LCH:(oc + 1) * LCH], h["ps"][:])
                                nc.sync.dma_start(
                                    out=aps["y"][lt * TB:(lt + 1) * TB,
                                                 oc * LCH:(oc + 1) * LCH],
                                    in_=h["osb"][:, oc * LCH:(oc + 1) * LCH])
                        fq_wo.append(wop)

        def attn_pair(p, i, prev_finish):
            """Causal attention for head-pair p over l-chunk i.

            Returns (ctxn_tile, finish_fn): finish_fn emits the deferred
            normalize multiplies (call it a few blocks into the NEXT pair so
            the broadcast DMA has completed before the DVE reaches them).
            """
            jmax = 4 * i + 3
            QTi = QT[p][i]
            ctxA = ctx_ps.tile([65, LCH], f32, tag="ctxA")
            ctxB = ctx_ps.tile([65, LCH], f32, tag="ctxB")
            pts = {}

            def sc_act(j):
                r = j - 4 * i
                off = 128 * r if r > 0 else 0
                sp = st_ps.tile([128, 2 * LCH], f32, tag="st")
                jn, jo = j // 4, (j % 4) * 128
                KTj = KT[p][jn]
                nc.tensor.matmul(sp[:, off:LCH], lhsT=KTj[0:64, jo:jo + 128],
                                 rhs=QTi[0:64, off:LCH], start=True, stop=True)
                nc.tensor.matmul(sp[:, LCH + off:2 * LCH], lhsT=KTj[64:128, jo:jo + 128],
                                 rhs=QTi[64:128, off:LCH], start=True, stop=True)
                pt = pt_pool.tile([128, 2 * LCH], bf16, tag="pt")
                nc.scalar.activation(pt[:, off:2 * LCH], sp[:, off:2 * LCH], Exp,
                                     scale=float(SCALE))
                if r >= 0:
                    # zero the causal triangle; PV never reads cols < off, so
                    # only the 128-wide diagonal slice needs masking
                    nc.vector.tensor_mul(pt[:, off:off + 128], pt[:, off:off + 128],
                                         masks[r][:, off:off + 128])
                    nc.vector.tensor_mul(pt[:, LCH + off:LCH + off + 128],
                                         pt[:, LCH + off:LCH + off + 128],
                                         masks[r][:, LCH + off:LCH + off + 128])
                pts[j] = (pt, off)

            def pv(j):
                pt, off = pts.pop(j)
                st_f = (j == 0)
                en = (j == jmax)
                nc.tensor.matmul(ctxA[:, off:LCH], lhsT=vt[j][:, 2 * p, :],
                                 rhs=pt[:, off:LCH], start=st_f, stop=en,
                                 skip_group_check=True)
                nc.tensor.matmul(ctxB[:, off:LCH], lhsT=vt[j][:, 2 * p + 1, :],
                                 rhs=pt[:, LCH + off:2 * LCH], start=st_f, stop=en,
                                 skip_group_check=True)

            sc_act(0)
            for j in range(jmax + 1):
                if j + 1 <= jmax:
                    sc_act(j + 1)
                draw(2)
                pv(j)
                state["blk"] += 1
                draw(2)
                if j == 2 and prev_finish is not None:
                    prev_finish()
                    prev_finish = None
            # drain ctx+sums into one SBUF tile (A cols 0:LCH, B cols LCH:)
            cpc = cpc_pool.tile([65, 2 * LCH], f32, tag=f"cpc{p}", name=f"cpc{p}")
            nc.vector.tensor_copy(cpc[:, 0:LCH], ctxA[:])
            nc.vector.tensor_copy(cpc[:, LCH:2 * LCH], ctxB[:])
            # move the sums row to partition 0 (HW partition_broadcast reads
            # partition 0 only), approx-recip there, broadcast to all partitions
            rec = small.tile([1, 2 * LCH], f32, tag="rec")
            nc.sync.dma_start(out=rec[:], in_=cpc[64:65, :])
            recr = small.tile([1, 2 * LCH], f32, tag="recr")
            nc.vector.reciprocal_approx_fast(out=recr[:], in_=rec[:])
            rb = small.tile([128, 2 * LCH], f32, tag="rb")
            nc.gpsimd.partition_broadcast(rb[:], recr[0:1, :])
            tn = ctxn_pool.tile([128, LCH], bf16, tag=f"cn{p}", name=f"cn{p}")

            def finish():
                tB = small.tile([64, LCH], bf16, tag="cnBtmp")
                nc.vector.tensor_mul(tn[0:64, :], cpc[0:64, 0:LCH], rb[0:64, 0:LCH])
                nc.vector.tensor_mul(tB[:], cpc[0:64, LCH:2 * LCH],
                                     rb[0:64, LCH:2 * LCH])
                nc.sync.dma_start(out=tn[64:128, :], in_=tB[:])

            return tn, finish

        def emit_x_dmas(n):
            xs = []
            for nm, ap in (("xv", aps["xvT"]), ("xq", aps["xqT"]), ("xk", aps["xkT"])):
                lst = []
                for c in range(NDC):
                    t = xpool.tile([128, LCH], bf16, tag=f"{nm}{c}", name=f"{nm}{c}_{n}")
                    nc.sync.dma_start(out=t[:], in_=ap[c * 128:(c + 1) * 128,
                                                       n * LCH:(n + 1) * LCH])
                    lst.append(t)
                xs.append(lst)
            return xs

        ctxn_prev = None
        prev_finish = None
        # chunk 0: V + pair-0 Q/K upfront, remaining pairs' Q/K as filler
        # gated per pair (spreads the startup lump to dodge the HAM throttle)
        xs0 = emit_x_dmas(0)
        gen_v(0, xs0[0])
        gen_qk(0, 0, xs0[1], xs0[2])
        flush(fq_proj)
        markers0 = {}
        for p in range(1, NP):
            gen_qk(0, p, xs0[1], xs0[2])
            markers0[p] = state["appended"]
        for n in range(NCH):
            # queue next chunk's projections + previous chunk's Wo as filler
            if n + 1 < NCH:
                xs = emit_x_dmas(n + 1)
                gen_v(n + 1, xs[0])
                for p in range(NP):
                    gen_qk(n + 1, p, xs[1], xs[2])
            if n > 0:
                gen_wo(n - 1, ctxn_prev)
            state["blk"] = 0
            ctxn_cur = [None] * NP
            for p in range(NP):
                if n == 0 and p > 0:
                    flush_to(markers0[p])
                ctxn_cur[p], prev_finish = attn_pair(p, n, prev_finish)
            flush(fq_proj)
            flush(fq_wo)
            ctxn_prev = ctxn_cur

        if prev_finish is not None:
            prev_finish()
        gen_wo(NCH - 1, ctxn_prev)
        flush(fq_wo)


def make_mask_tiles(cfg):
    T_, LCH_, TB_ = cfg["T"], min(512, cfg["T"]), 128
    nMask = LCH_ // TB_
    f = np.arange(2 * LCH_) % LCH_
    p = np.arange(TB_)
    tiles = []
    for r in range(nMask):
        m = (f[None, :] >= (TB_ * r + p)[:, None]).astype(np.float32)
        tiles.append(m)
    return np.stack(tiles).astype(BF16)


def build_nc(cfg):
    """Build and compile the per-core Bass program. Returns nc."""
    import concourse.bacc as bacc
    import concourse.tile as tile
    from concourse import mybir

    T_, DM_, DG_ = cfg["T"], cfg["DM"], cfg["DG"]

    nc = bacc.Bacc("TRN2", target_bir_lowering=False, debug=False)
    f32 = mybir.dt.float32
    bf16 = mybir.dt.bfloat16
    aps = {}
    for nm, shape, dt in [
        ("xqT", [DM_, T_], bf16), ("xkT", [DM_, T_], bf16), ("xvT", [DM_, T_], bf16),
        ("wqT", [DM_, DG_], bf16), ("wkT", [DM_, DG_], bf16), ("wvT", [DM_, DG_], bf16),
        ("woT", [DG_, DM_], bf16),
        ("maskt", [4, 128, 1024], bf16),
    ]:
        aps[nm] = nc.dram_tensor(nm, shape, dt, kind="ExternalInput").ap()
    aps["y"] = nc.dram_tensor("y", [T_, DM_], f32, kind="ExternalOutput").ap()

    with tile.TileContext(nc) as tc:
        emit_mha(tc, aps)
    nc.compile()
    return nc


_CACHE = {}


def _get_nc():
    if "nc" not in _CACHE:
        _CACHE["nc"] = build_nc(FULL_CFG)
    return _CACHE["nc"]


def shard_inputs(q, k, v, Wq, Wk, Wv, Wo):
    """Build the per-core input maps (8 cores = 4 batches x 2 groups)."""
    maskt = make_mask_tiles(FULL_CFG)
    xT = {}
    for b in range(B):
        xT[b] = (np.ascontiguousarray(q[b].T).astype(BF16),
                 np.ascontiguousarray(k[b].T).astype(BF16),
                 np.ascontiguousarray(v[b].T).astype(BF16))
    wT = {}
    for g in range(GROUPS):
        rows = slice(g * DG, (g + 1) * DG)
        wT[g] = (np.ascontiguousarray(Wq[rows].T).astype(BF16),
                 np.ascontiguousarray(Wk[rows].T).astype(BF16),
                 np.ascontiguousarray(Wv[rows].T).astype(BF16),
                 np.ascontiguousarray(Wo[:, rows].T).astype(BF16))
    in_maps = []
    for core in range(NCORES):
        b, g = divmod(core, GROUPS)
        xqT, xkT, xvT = xT[b]
        wqT, wkT, wvT, woT = wT[g]
        in_maps.append({
            "xqT": xqT, "xkT": xkT, "xvT": xvT,
            "wqT": wqT, "wkT": wkT, "wvT": wvT, "woT": woT,
            "maskt": maskt,
        })
    return in_maps


def kernel(q, k, v, mask, Wq, Wk, Wv, Wo):
    from concourse import bass_utils

    q = np.asarray(q, dtype=np.float32)
    k = np.asarray(k, dtype=np.float32)
    v = np.asarray(v, dtype=np.float32)
    Wq = np.asarray(Wq, dtype=np.float32)
    Wk = np.asarray(Wk, dtype=np.float32)
    Wv = np.asarray(Wv, dtype=np.float32)
    Wo = np.asarray(Wo, dtype=np.float32)

    nc = _get_nc()
    in_maps = shard_inputs(q, k, v, Wq, Wk, Wv, Wo)
    res = bass_utils.run_bass_kernel_spmd(nc, in_maps, core_ids=list(range(NCORES)))
    out = np.zeros((B, L, D), dtype=np.float32)
    for core in range(NCORES):
        b = core // GROUPS
        out[b] += res.results[core]["y"]
    return out


# revision 33
# speedup vs baseline: 1.0037x; 1.0019x over previous
"""Trainium2 Bass kernel for nn_MultiHeadAttention (B=4, L=S=2048, D=1024, H=16, causal).

Sharding: 8 cores = 4 batches x 2 head-groups (8 heads each).
Per core: project its batch's q/k/v against its group's weight slices,
causal attention for 8 heads, output-projection against Wo column slice.
Host sums the 2 partial outputs per batch (tensor-parallel reduce).

v2 layout notes:
- All input transposes happen on the host (numpy), so every device DMA is
  linear. X arrives as xT [D, T] per tensor; weights pre-transposed.
- Softmax sums are folded into the PV matmul: each head's V tile carries a
  65th column of ones, so ctx PSUM row 64 accumulates sum(P) for free.
- Scores for diagonal blocks are tightened to the causal width; the mask
  multiply zeroes the stale/garbage columns (masks have 0 there).
- Projections, attention and Wo are interleaved chunk-by-chunk so the PE
  instruction stream never drains.

All matmuls bf16 with fp32 PSUM accumulation.
"""

import sys

if "/opt/trn_rl_repo" not in sys.path:
    sys.path.insert(0, "/opt/trn_rl_repo")

import numpy as np
import ml_dtypes

BF16 = ml_dtypes.bfloat16

# Problem constants (hardcoded per harness contract)
B, L, D, H = 4, 2048, 1024, 16
HD = D // H              # 64
NCORES = 8
GROUPS = 2               # head-groups (tensor parallel)
HG = H // GROUPS         # 8 heads per group
DG = HG * HD             # 512 out-dim per group

T = L                    # tokens per core
DM = D                   # model dim
NDC = DM // 128          # 8 contraction chunks
NP = HG // 2             # 4 head pairs
NCH = T // 512           # 4 token chunks
LCH = 512
TB = 128
NT = T // TB             # 16 token tiles
SCALE = 1.0 / np.sqrt(HD)

FULL_CFG = dict(T=T, DM=DM, DG=DG)


def emit_mha(tc, aps):
    import concourse.bass as bass
    from concourse import mybir

    nc = tc.nc
    f32 = mybir.dt.float32
    bf16 = mybir.dt.bfloat16
    Exp = mybir.ActivationFunctionType.Exp

    import contextlib

    ctx = contextlib.ExitStack()
    with ctx:
        wts = ctx.enter_context(tc.tile_pool(name="wts", bufs=1))
        xpool = ctx.enter_context(tc.tile_pool(name="xp", bufs=2))
        vt_pool = ctx.enter_context(tc.tile_pool(name="vt", bufs=1))
        qt_pool = ctx.enter_context(tc.tile_pool(name="qt", bufs=1))
        kt_pool = ctx.enter_context(tc.tile_pool(name="kt", bufs=1))
        pt_pool = ctx.enter_context(tc.tile_pool(name="ptp", bufs=4))
        cpc_pool = ctx.enter_context(tc.tile_pool(name="cpc", bufs=1))
        ctxn_pool = ctx.enter_context(tc.tile_pool(name="ctxn", bufs=2))
        small = ctx.enter_context(tc.tile_pool(name="small", bufs=1))
        osb_pool = ctx.enter_context(tc.tile_pool(name="osb", bufs=2))
        # PSUM: scores 2x2 banks + ctxA 1 + ctxB 1 + proj 2 = 8 banks
        st_ps = ctx.enter_context(tc.tile_pool(name="st_ps", bufs=2, space="PSUM"))
        ctx_ps = ctx.enter_context(tc.tile_pool(name="ctx_ps", bufs=1, space="PSUM"))
        proj_ps = ctx.enter_context(tc.tile_pool(name="proj_ps", bufs=2, space="PSUM"))

        # ---- weight/mask DMAs on the Activation HWDGE queue ----
        wvT, wqT, wkT = [], [], []
        for nm, lst in (("wv", wvT), ("wq", wqT), ("wk", wkT)):
            for c in range(NDC):
                t = wts.tile([128, DG], bf16, tag=f"{nm}{c}", name=f"{nm}{c}")
                nc.scalar.dma_start(out=t[:], in_=aps[nm + "T"][c * 128:(c + 1) * 128, :])
                lst.append(t)
        woT2 = []
        for p in range(NP):
            t = wts.tile([128, DM], bf16, tag=f"wo{p}", name=f"wo{p}")
            nc.scalar.dma_start(out=t[:], in_=aps["woT"][p * 128:(p + 1) * 128, :])
            woT2.append(t)
        masks = []
        for r in range(4):
            mt = wts.tile([TB, 2 * LCH], bf16, tag=f"mask{r}", name=f"mask{r}")
            nc.scalar.dma_start(out=mt[:], in_=aps["maskt"][r])
            masks.append(mt)

        # pre-zero the pt buffers (stale cols are mask-multiplied; NaN*0=NaN)
        pt_boot = []
        for _ in range(4):
            pt = pt_pool.tile([128, 2 * LCH], bf16, tag="pt")
            nc.vector.memset(pt[:], 0.0)
            pt_boot.append(pt)
        # pre-zero score PSUM so exp of untouched columns stays finite
        for _ in range(2):
            sp = st_ps.tile([128, 2 * LCH], f32, tag="st")
            nc.vector.memset(sp[:], 0.0)

        vt = [None] * NT            # [128, HG, HD+1] V tiles (ones in col HD)
        QT = [[None] * NCH for _ in range(NP)]
        KT = [[None] * NCH for _ in range(NP)]

        # ---- filler queues: projection/Wo matmuls emitted one-at-a-time
        # between attention blocks so the PE stream never drains ----
        fq_proj, fq_wo = [], []
        state = {"blk": 0, "popped": 0, "appended": 0}

        def push_proj(fn):
            fq_proj.append(fn)
            state["appended"] += 1

        def draw(k):
            for _ in range(k):
                if fq_proj:
                    fq_proj.pop(0)()
                    state["popped"] += 1
                elif fq_wo and state["blk"] >= 8:
                    fq_wo.pop(0)()
                else:
                    return

        def flush(q):
            while q:
                q.pop(0)()
                if q is fq_proj:
                    state["popped"] += 1

        def flush_to(marker):
            while state["popped"] < marker:
                fq_proj.pop(0)()
                state["popped"] += 1

        def gen_v(n, xv_n):
            for stl in range(4):
                st = 4 * n + stl
                h = {}
                for c in range(NDC):
                    def vop(c=c, stl=stl, st=st, h=h, xv_n=xv_n):
                        if c == 0:
                            h["ps"] = proj_ps.tile([128, DG], f32, tag="proj", name="pps")
                        nc.tensor.matmul(h["ps"][:],
                                         lhsT=xv_n[c][:, stl * 128:(stl + 1) * 128],
                                         rhs=wvT[c][:], start=(c == 0),
                                         stop=(c == NDC - 1))
                        if c == NDC - 1:
                            v = vt_pool.tile([128, HG, HD + 1], bf16, tag=f"V{st}",
                                             name=f"V{st}")
                            nc.vector.tensor_copy(
                                v[:, :, 0:HD],
                                h["ps"][:].rearrange("a (b c) -> a b c", b=HG))
                            nc.vector.memset(v[:, :, HD:HD + 1], 1.0)
                            vt[st] = v
                    push_proj(vop)

        def gen_qk(n, p, xq_n, xk_n):
                hq, hk = {}, {}
                for c in range(NDC):
                    def qop(c=c, p=p, n=n, h=hq, xq_n=xq_n):
                        if c == 0:
                            h["ps"] = proj_ps.tile([128, LCH], f32, tag="proj", name="pps")
                        nc.tensor.matmul(h["ps"][:],
                                         lhsT=wqT[c][:, p * 128:(p + 1) * 128],
                                         rhs=xq_n[c][:], start=(c == 0),
                                         stop=(c == NDC - 1))
                        if c == NDC - 1:
                            qt = qt_pool.tile([128, LCH], bf16, tag=f"QT{p}_{n}",
                                              name=f"QT{p}_{n}")
                            nc.vector.tensor_copy(qt[:], h["ps"][:])
                            QT[p][n] = qt
                    push_proj(qop)
                for c in range(NDC):
                    def kop(c=c, p=p, n=n, h=hk, xk_n=xk_n):
                        if c == 0:
                            h["ps"] = proj_ps.tile([128, LCH], f32, tag="proj", name="pps")
                        nc.tensor.matmul(h["ps"][:],
                                         lhsT=wkT[c][:, p * 128:(p + 1) * 128],
                                         rhs=xk_n[c][:], start=(c == 0),
                                         stop=(c == NDC - 1))
                        if c == NDC - 1:
                            kt = kt_pool.tile([128, LCH], bf16, tag=f"KT{p}_{n}",
                                              name=f"KT{p}_{n}")
                            nc.vector.tensor_copy(kt[:], h["ps"][:])
                            KT[p][n] = kt
                    push_proj(kop)

        def gen_wo(m, ctxn):
            for ltl in range(4):
                lt = 4 * m + ltl
                h = {}
                for oc in range(2):
                    for p in range(NP):
                        def wop(ltl=ltl, lt=lt, oc=oc, p=p, h=h, ctxn=ctxn):
                            if oc == 0 and p == 0:
                                h["osb"] = osb_pool.tile([128, DM], f32, tag="osb", name="osb")
                            if p == 0:
                                h["ps"] = proj_ps.tile([128, LCH], f32, tag="proj", name="pps")
                            nc.tensor.matmul(
                                h["ps"][:],
                                lhsT=ctxn[p][:, ltl * 128:(ltl + 1) * 128],
                                rhs=woT2[p][:, oc * LCH:(oc + 1) * LCH],
                                start=(p == 0), stop=(p == NP - 1))
                            if p == NP - 1:
                                nc.vector.tensor_copy(
                                    h["osb"][:, oc * LCH:(oc + 1) * LCH], h["ps"][:])
                                nc.sync.dma_start(
                                    out=aps["y"][lt * TB:(lt + 1) * TB,
                                                 oc * LCH:(oc + 1) * LCH],
                                    in_=h["osb"][:, oc * LCH:(oc + 1) * LCH])
                        fq_wo.append(wop)

        def attn_pair(p, i, prev_finish):
            """Causal attention for head-pair p over l-chunk i.

            Returns (ctxn_tile, finish_fn): finish_fn emits the deferred
            normalize multiplies (call it a few blocks into the NEXT pair so
            the broadcast DMA has completed before the DVE reaches them).
            """
            jmax = 4 * i + 3
            QTi = QT[p][i]
            ctxA = ctx_ps.tile([65, LCH], f32, tag="ctxA")
            ctxB = ctx_ps.tile([65, LCH], f32, tag="ctxB")
            pts = {}

            def sc_act(j):
                r = j - 4 * i
                off = 128 * r if r > 0 else 0
                sp = st_ps.tile([128, 2 * LCH], f32, tag="st")
                jn, jo = j // 4, (j % 4) * 128
                KTj = KT[p][jn]
                nc.tensor.matmul(sp[:, off:LCH], lhsT=KTj[0:64, jo:jo + 128],
                                 rhs=QTi[0:64, off:LCH], start=True, stop=True)
                nc.tensor.matmul(sp[:, LCH + off:2 * LCH], lhsT=KTj[64:128, jo:jo + 128],
                                 rhs=QTi[64:128, off:LCH], start=True, stop=True)
                pt = pt_pool.tile([128, 2 * LCH], bf16, tag="pt")
                nc.scalar.activation(pt[:, off:2 * LCH], sp[:, off:2 * LCH], Exp,
                                     scale=float(SCALE))
                if r >= 0:
                    # zero the causal triangle; PV never reads cols < off, so
                    # only the 128-wide diagonal slice needs masking
                    nc.vector.tensor_mul(pt[:, off:off + 128], pt[:, off:off + 128],
                                         masks[r][:, off:off + 128])
                    nc.vector.tensor_mul(pt[:, LCH + off:LCH + off + 128],
                                         pt[:, LCH + off:LCH + off + 128],
                                         masks[r][:, LCH + off:LCH + off + 128])
                pts[j] = (pt, off)

            def pv(j):
                pt, off = pts.pop(j)
                st_f = (j == 0)
                en = (j == jmax)
                nc.tensor.matmul(ctxA[:, off:LCH], lhsT=vt[j][:, 2 * p, :],
                                 rhs=pt[:, off:LCH], start=st_f, stop=en,
                                 skip_group_check=True)
                nc.tensor.matmul(ctxB[:, off:LCH], lhsT=vt[j][:, 2 * p + 1, :],
                                 rhs=pt[:, LCH + off:2 * LCH], start=st_f, stop=en,
                                 skip_group_check=True)

            sc_act(0)
            for j in range(jmax + 1):
                if j + 1 <= jmax:
                    sc_act(j + 1)
                draw(3)
                pv(j)
                state["blk"] += 1
                draw(3)
                if j == 2 and prev_finish is not None:
                    prev_finish()
                    prev_finish = None
            # drain ctx+sums into one SBUF tile (A cols 0:LCH, B cols LCH:)
            cpc = cpc_pool.tile([65, 2 * LCH], f32, tag=f"cpc{p}", name=f"cpc{p}")
            nc.vector.tensor_copy(cpc[:, 0:LCH], ctxA[:])
            nc.vector.tensor_copy(cpc[:, LCH:2 * LCH], ctxB[:])
            # move the sums row to partition 0 (HW partition_broadcast reads
            # partition 0 only), approx-recip there, broadcast to all partitions
            rec = small.tile([1, 2 * LCH], f32, tag="rec")
            nc.sync.dma_start(out=rec[:], in_=cpc[64:65, :])
            recr = small.tile([1, 2 * LCH], f32, tag="recr")
            nc.vector.reciprocal_approx_fast(out=recr[:], in_=rec[:])
            rb = small.tile([128, 2 * LCH], f32, tag="rb")
            nc.gpsimd.partition_broadcast(rb[:], recr[0:1, :])
            tn = ctxn_pool.tile([128, LCH], bf16, tag=f"cn{p}", name=f"cn{p}")

            def finish():
                tB = small.tile([64, LCH], bf16, tag="cnBtmp")
                nc.vector.tensor_mul(tn[0:64, :], cpc[0:64, 0:LCH], rb[0:64, 0:LCH])
                nc.vector.tensor_mul(tB[:], cpc[0:64, LCH:2 * LCH],
                                     rb[0:64, LCH:2 * LCH])
                nc.sync.dma_start(out=tn[64:128, :], in_=tB[:])

            return tn, finish

        def emit_x_dmas(n):
            xs = []
            for nm, ap in (("xv", aps["xvT"]), ("xq", aps["xqT"]), ("xk", aps["xkT"])):
                lst = []
                for c in range(NDC):
                    t = xpool.tile([128, LCH], bf16, tag=f"{nm}{c}", name=f"{nm}{c}_{n}")
                    nc.sync.dma_start(out=t[:], in_=ap[c * 128:(c + 1) * 128,
                                                       n * LCH:(n + 1) * LCH])
                    lst.append(t)
                xs.append(lst)
            return xs

        ctxn_prev = None
        prev_finish = None
        # chunk 0: V + pair-0 Q/K upfront, remaining pairs' Q/K as filler
        # gated per pair (spreads the startup lump to dodge the HAM throttle)
        xs0 = emit_x_dmas(0)
        gen_v(0, xs0[0])
        gen_qk(0, 0, xs0[1], xs0[2])
        flush(fq_proj)
        markers0 = {}
        for p in range(1, NP):
            gen_qk(0, p, xs0[1], xs0[2])
            markers0[p] = state["appended"]
        for n in range(NCH):
            # queue next chunk's projections + previous chunk's Wo as filler
            if n + 1 < NCH:
                xs = emit_x_dmas(n + 1)
                gen_v(n + 1, xs[0])
                for p in range(NP):
                    gen_qk(n + 1, p, xs[1], xs[2])
            if n > 0:
                gen_wo(n - 1, ctxn_prev)
            state["blk"] = 0
            ctxn_cur = [None] * NP
            for p in range(NP):
                if n == 0 and p > 0:
                    flush_to(markers0[p])
                ctxn_cur[p], prev_finish = attn_pair(p, n, prev_finish)
            flush(fq_proj)
            flush(fq_wo)
            ctxn_prev = ctxn_cur

        if prev_finish is not None:
            prev_finish()
        gen_wo(NCH - 1, ctxn_prev)
        flush(fq_wo)


def make_mask_tiles(cfg):
    T_, LCH_, TB_ = cfg["T"], min(512, cfg["T"]), 128
    nMask = LCH_ // TB_
    f = np.arange(2 * LCH_) % LCH_
    p = np.arange(TB_)
    tiles = []
    for r in range(nMask):
        m = (f[None, :] >= (TB_ * r + p)[:, None]).astype(np.float32)
        tiles.append(m)
    return np.stack(tiles).astype(BF16)


def build_nc(cfg):
    """Build and compile the per-core Bass program. Returns nc."""
    import concourse.bacc as bacc
    import concourse.tile as tile
    from concourse import mybir

    T_, DM_, DG_ = cfg["T"], cfg["DM"], cfg["DG"]

    nc = bacc.Bacc("TRN2", target_bir_lowering=False, debug=False)
    f32 = mybir.dt.float32
    bf16 = mybir.dt.bfloat16
    aps = {}
    for nm, shape, dt in [
        ("xqT", [DM_, T_], bf16), ("xkT", [DM_, T_], bf16), ("xvT", [DM_, T_], bf16),
        ("wqT", [DM_, DG_], bf16), ("wkT", [DM_, DG_], bf16), ("wvT", [DM_, DG_], bf16),
        ("woT", [DG_, DM_], bf16),
        ("maskt", [4, 128, 1024], bf16),
    ]:
        aps[nm] = nc.dram_tensor(nm, shape, dt, kind="ExternalInput").ap()
    aps["y"] = nc.dram_tensor("y", [T_, DM_], f32, kind="ExternalOutput").ap()

    with tile.TileContext(nc) as tc:
        emit_mha(tc, aps)
    nc.compile()
    return nc


_CACHE = {}


def _get_nc():
    if "nc" not in _CACHE:
        _CACHE["nc"] = build_nc(FULL_CFG)
    return _CACHE["nc"]


def shard_inputs(q, k, v, Wq, Wk, Wv, Wo):
    """Build the per-core input maps (8 cores = 4 batches x 2 groups)."""
    maskt = make_mask_tiles(FULL_CFG)
    xT = {}
    for b in range(B):
        xT[b] = (np.ascontiguousarray(q[b].T).astype(BF16),
                 np.ascontiguousarray(k[b].T).astype(BF16),
                 np.ascontiguousarray(v[b].T).astype(BF16))
    wT = {}
    for g in range(GROUPS):
        rows = slice(g * DG, (g + 1) * DG)
        wT[g] = (np.ascontiguousarray(Wq[rows].T).astype(BF16),
                 np.ascontiguousarray(Wk[rows].T).astype(BF16),
                 np.ascontiguousarray(Wv[rows].T).astype(BF16),
                 np.ascontiguousarray(Wo[:, rows].T).astype(BF16))
    in_maps = []
    for core in range(NCORES):
        b, g = divmod(core, GROUPS)
        xqT, xkT, xvT = xT[b]
        wqT, wkT, wvT, woT = wT[g]
        in_maps.append({
            "xqT": xqT, "xkT": xkT, "xvT": xvT,
            "wqT": wqT, "wkT": wkT, "wvT": wvT, "woT": woT,
            "maskt": maskt,
        })
    return in_maps


def kernel(q, k, v, mask, Wq, Wk, Wv, Wo):
    from concourse import bass_utils

    q = np.asarray(q, dtype=np.float32)
    k = np.asarray(k, dtype=np.float32)
    v = np.asarray(v, dtype=np.float32)
    Wq = np.asarray(Wq, dtype=np.float32)
    Wk = np.asarray(Wk, dtype=np.float32)
    Wv = np.asarray(Wv, dtype=np.float32)
    Wo = np.asarray(Wo, dtype=np.float32)

    nc = _get_nc()
    in_maps = shard_inputs(q, k, v, Wq, Wk, Wv, Wo)
    res = bass_utils.run_bass_kernel_spmd(nc, in_maps, core_ids=list(range(NCORES)))
    out = np.zeros((B, L, D), dtype=np.float32)
    for core in range(NCORES):
        b = core // GROUPS
        out[b] += res.results[core]["y"]
    return out
